# revision 44
# baseline (speedup 1.0000x reference)
"""GATv2 block (GAT conv + head-mean + BatchNorm + ReLU) on 8 Trainium2 cores.

Sharding: nodes split contiguously across 8 cores (graph/data parallel).
Edges (incl. self loops) are bucketed by destination core and 128-node
destination block, so segment-softmax and the scatter-add stay core-local.
Every core computes the full xl = x @ W_l so the per-edge gathers of
xl[src] are local dma_gathers.  BN batch stats do one AllReduce of
[128, 2] partial sums.

Per 128-edge tile (d = feature dim on partitions for the score path, via a
TRANSPOSING dma_gather of xl[src]):
  wT[d,(h,e)] = xr_bh.T @ SbT  +  I.T @ xlgT          (PE, into PSUM)
  mT  = leaky_relu(wT)                                 (ACT Prelu)
  s[h,e] = attT_h.T @ mT_h      -- score dot on PE (one [1,128] mm per head)
  -- per 16-tile group: ee = exp(s) (one ACT op), ee.T via one PE transpose,
     ee_sb/eef copies (DVE), den += S_t.T @ ee (PE)
  out += S_t.T @ (eef * xlg)                           (DVE ts + PE)
then per node block: out /= den, head-sum (head-mean folds into BN with eps
scaled by H^2), BN partials via ones-matmul.
"""

import math

import numpy as np

HEADS = 4
HIDDEN = 128
NEG_SLOPE = 0.2
BN_EPS = 1e-5
NCORES = 8

_cache = {}


# --------------------------------------------------------------------------
# Host-side preprocessing
# --------------------------------------------------------------------------

def _prep_host(x, edge_index, W_l, b_l, W_r, b_r, att, bias, gamma, beta):
    import ml_dtypes

    N, C = x.shape
    H, D = att.shape
    HD = H * D
    NL = N // NCORES                      # local nodes per core
    NB = (NL + 127) // 128                # node blocks per core
    NLpad = NB * 128
    Npad = ((N + 127) // 128) * 128

    src = np.concatenate([np.asarray(edge_index[0]), np.arange(N)]).astype(np.int64)
    dst = np.concatenate([np.asarray(edge_index[1]), np.arange(N)]).astype(np.int64)

    core_of = dst // NL
    # Degree-balanced node->block assignment within each core (greedy LPT):
    # equalizes per-block edge counts so the uniform tiles-per-block T is
    # close to the mean instead of the max.  perm[k][j] = original local id
    # of the node placed at padded-local slot j.
    edge_src = [[None] * NB for _ in range(NCORES)]
    perm = np.zeros((NCORES, NLpad), np.int64)
    for k in range(NCORES):
        sel = core_of == k
        s_k = src[sel]
        d_k = dst[sel] - k * NL
        deg = np.bincount(d_k, minlength=NL)
        order = np.argsort(-deg, kind="stable")
        blk_of = np.zeros(NL, np.int64)
        slot_of = np.zeros(NL, np.int64)
        loads = np.zeros(NB, np.int64)
        fill = np.zeros(NB, np.int64)
        cap = [128] * (NB - 1) + [128 - (NLpad - NL)]
        for n in order:
            cands = np.nonzero(fill < cap)[0]
            b = cands[np.argmin(loads[cands])]
            blk_of[n] = b
            slot_of[n] = fill[b]
            loads[b] += deg[n]
            fill[b] += 1
        for b in range(NB):
            members = np.nonzero(blk_of == b)[0]
            perm[k, b * 128: b * 128 + len(members)] = \
                members[np.argsort(slot_of[members])]
        d_loc = blk_of[d_k] * 128 + slot_of[d_k]   # padded-local slot of dst
        blk = d_loc // 128
        order_e = np.argsort(blk, kind="stable")
        s_k, d_loc, blk = s_k[order_e], d_loc[order_e], blk[order_e]
        bounds = np.searchsorted(blk, np.arange(NB + 1))
        for b in range(NB):
            lo, hi = bounds[b], bounds[b + 1]
            edge_src[k][b] = (s_k[lo:hi], d_loc[lo:hi] - b * 128)

    n_fake_last = NLpad - NL
    T = 1
    for k in range(NCORES):
        for b in range(NB):
            cnt = len(edge_src[k][b][0])
            extra = n_fake_last if b == NB - 1 else 0
            T = max(T, (cnt + extra + 127) // 128)
    ET = T * 128

    gidx = np.zeros((NCORES, NB, 128, ET // 16), np.int16)
    S_t = np.zeros((NCORES, NB, 128, ET), ml_dtypes.float8_e4m3)
    S_bT = np.zeros((NCORES, NB, 128, ET), ml_dtypes.float8_e4m3)
    for k in range(NCORES):
        for b in range(NB):
            s_e, d_e = edge_src[k][b]
            cnt = len(s_e)
            sidx = np.zeros(ET, np.int64)
            sidx[:cnt] = s_e
            dloc = np.full(ET, -1, np.int64)
            dloc[:cnt] = d_e
            if b == NB - 1 and n_fake_last:
                fake = np.arange(128 - n_fake_last, 128)
                assert cnt + n_fake_last <= ET, "pad shortage for fake nodes"
                dloc[cnt:cnt + n_fake_last] = fake
            # wrapped int16 layout: idx i -> [i % 16, i // 16], replicated
            # down all 8 groups of 16 partitions
            w = sidx.reshape(ET // 16, 16).T.astype(np.int16)
            gidx[k, b] = np.tile(w, (8, 1))
            e_ids = np.arange(ET)
            t_id, e_p = e_ids // 128, e_ids % 128
            valid = dloc >= 0
            S_t[k, b, e_p[valid], t_id[valid] * 128 + dloc[valid]] = 1.0
            S_bT[k, b, dloc[valid], t_id[valid] * 128 + e_p[valid]] = 1.0

    ones_m = np.zeros((128, NB), np.float32)
    for b in range(NB):
        ones_m[: max(0, min(128, NL - b * 128)), b] = 1.0

    xT = np.zeros((C, Npad), ml_dtypes.bfloat16)
    xT[:, :N] = np.asarray(x, np.float32).T.astype(ml_dtypes.bfloat16)
    xT_loc = np.zeros((NCORES, C, NLpad), ml_dtypes.bfloat16)
    xfull = np.asarray(x, np.float32)
    valid_slot = np.zeros(NLpad, bool)
    for b in range(NB):
        cap_b = 128 if b < NB - 1 else 128 - (NLpad - NL)
        valid_slot[b * 128: b * 128 + cap_b] = True
    for k in range(NCORES):
        cols = xfull[k * NL + perm[k]].T          # [C, NLpad], permuted
        cols[:, ~valid_slot] = 0.0
        xT_loc[k] = cols.astype(ml_dtypes.bfloat16)

    b_l = np.asarray(b_l, np.float32)
    b_sum = b_l + np.asarray(b_r, np.float32)
    has_b = bool(np.any(b_sum != 0) or np.any(b_l != 0))

    return dict(
        N=N, C=C, H=H, D=D, HD=HD, NL=NL, NB=NB, NLpad=NLpad, Npad=Npad,
        T=T, ET=ET, has_b=has_b,
        W_l=np.asarray(W_l, np.float32).astype(ml_dtypes.bfloat16),
        W_r=np.asarray(W_r, np.float32).astype(ml_dtypes.bfloat16),
        attT_col=np.ascontiguousarray(np.asarray(att, np.float32).T),  # [D, H]
        attPad0=np.concatenate(
            [np.asarray(att, np.float32).T, np.zeros((D, 28), np.float32)],
            axis=1),                                   # [D, 32] att|zeros
        attPadZ=np.concatenate(
            [np.zeros((D, 32), np.float32), np.asarray(att, np.float32).T],
            axis=1),                                   # [D, 36] zeros|att
        bsum_rep=np.broadcast_to(b_sum.reshape(1, HD), (128, HD)).copy(),
        bl_rep=np.broadcast_to(b_l.reshape(1, HD), (128, HD)).copy(),
        gamma_col=np.asarray(gamma, np.float32).reshape(D, 1),
        beta_col=np.asarray(beta, np.float32).reshape(D, 1),
        epsp_col=np.full((D, 1), BN_EPS * H * H, np.float32),
        xT=xT, xT_loc=xT_loc, ones_m=ones_m,
        gidx=gidx, S_t=S_t, S_bT=S_bT, perm=perm, valid_slot=valid_slot,
    )


# --------------------------------------------------------------------------
# Device program
# --------------------------------------------------------------------------

def _build_nc(hp, debug=False, no_cc=False, GSZ=8):
    import concourse.bacc as bacc
    import concourse.bass as bass
    import concourse.tile as tile
    from concourse import mybir
    from concourse.library_config import mlp
    from concourse.masks import make_identity

    dt = mybir.dt
    AF = mybir.ActivationFunctionType
    ALU = mybir.AluOpType

    N, C, H, D, HD = hp["N"], hp["C"], hp["H"], hp["D"], hp["HD"]
    NL, NB, NLpad, Npad = hp["NL"], hp["NB"], hp["NLpad"], hp["Npad"]
    T, ET, has_b = hp["T"], hp["ET"], hp["has_b"]
    NXC = Npad // 128
    f32r = dt.float32r

    nc = bacc.Bacc(
        "TRN2", target_bir_lowering=False, debug=debug, num_devices=NCORES
    )

    # ---- I/O ----
    t_xT = nc.dram_tensor("xT", [C, Npad], dt.bfloat16, kind="ExternalInput")
    t_xT_loc = nc.dram_tensor("xT_loc", [C, NLpad], dt.bfloat16, kind="ExternalInput")
    t_Wl = nc.dram_tensor("W_l", [C, HD], dt.bfloat16, kind="ExternalInput")
    t_Wr = nc.dram_tensor("W_r", [C, HD], dt.bfloat16, kind="ExternalInput")
    t_attT = nc.dram_tensor("attT_col", [D, H], dt.float32, kind="ExternalInput")
    t_attP0 = nc.dram_tensor("attPad0", [D, 32], dt.float32, kind="ExternalInput")
    t_attPZ = nc.dram_tensor("attPadZ", [D, 36], dt.float32, kind="ExternalInput")
    if has_b:
        t_bsum = nc.dram_tensor("bsum_rep", [128, HD], dt.float32,
                                kind="ExternalInput")
        t_bl = nc.dram_tensor("bl_rep", [128, HD], dt.float32,
                              kind="ExternalInput")
    t_gamma = nc.dram_tensor("gamma_col", [D, 1], dt.float32, kind="ExternalInput")
    t_beta = nc.dram_tensor("beta_col", [D, 1], dt.float32, kind="ExternalInput")
    t_epsp = nc.dram_tensor("epsp_col", [D, 1], dt.float32, kind="ExternalInput")
    t_ones = nc.dram_tensor("ones_m", [128, NB], dt.float32, kind="ExternalInput")
    t_gidx = nc.dram_tensor("gidx", [NB, 128, ET // 16], dt.int16,
                            kind="ExternalInput")
    t_St = nc.dram_tensor("S_t", [NB, 128, ET], dt.float8e4, kind="ExternalInput")
    t_SbT = nc.dram_tensor("S_bT", [NB, 128, ET], dt.float8e4, kind="ExternalInput")
    t_y = nc.dram_tensor("y", [NLpad, D], dt.float32, kind="ExternalOutput")

    t_xl = nc.dram_tensor("xl_scratch", [Npad, HD], dt.bfloat16)
    t_ccin = nc.dram_tensor("cc_in", [D, 2], dt.float32)
    t_ccout = nc.dram_tensor("cc_out", [D, 2], dt.float32)

    # tile groups for the score pack: GSZ tiles -> spack [4*GSZ, 128]
    groups = []
    t0 = 0
    while t0 < T:
        groups.append((t0, min(T, t0 + GSZ)))
        t0 += GSZ

    with tile.TileContext(nc) as tc:
        nc.gpsimd.load_library(mlp)

        with tc.tile_pool(name="consts", bufs=1) as consts, \
             tc.tile_pool(name="persist", bufs=1) as persist, \
             tc.tile_pool(name="statp", bufs=1, space="PSUM") as statp:

            wl_sb = consts.tile([C, HD], dt.bfloat16)
            nc.sync.dma_start(wl_sb[:], t_Wl[:, :])
            wr_sb = consts.tile([C, HD], dt.bfloat16)
            nc.sync.dma_start(wr_sb[:], t_Wr[:, :])
            attP0_sb = consts.tile([D, 4 * GSZ], dt.bfloat16)
            nc.gpsimd.dma_start(attP0_sb[:], t_attP0[:, :])
            attPZ_sb = consts.tile([D, 4 * GSZ + 4], dt.bfloat16)
            nc.gpsimd.dma_start(attPZ_sb[:], t_attPZ[:, :])
            if has_b:
                bsum_sb = consts.tile([128, HD], dt.float32)
                nc.sync.dma_start(bsum_sb[:], t_bsum[:, :])
                bl_sb = consts.tile([128, HD], dt.float32)
                nc.sync.dma_start(bl_sb[:], t_bl[:, :])
            ident_bf = consts.tile([128, 128], dt.bfloat16)
            make_identity(nc, ident_bf[:])
            ident_f32 = consts.tile([128, 128], dt.float32)
            make_identity(nc, ident_f32[:])
            zeros2 = consts.tile([128, 2], dt.float32)
            nc.vector.memset(zeros2[:], 0.0)
            zeros2b = consts.tile([128, 2], dt.bfloat16)
            nc.vector.memset(zeros2b[:], 0.0)

            xr_all = persist.tile([128, NB, HD], dt.bfloat16)
            om_all = persist.tile([128, NB, D], dt.float32)
            stat_ps = statp.tile([D, 2], dt.float32, space="PSUM")

            # ---- xl = x @ W_l (all nodes); xr = x_local @ W_r ----
            with tc.tile_pool(name="xtc", bufs=2) as xtcp, \
                 tc.tile_pool(name="xlps", bufs=2, space="PSUM") as xlpsp, \
                 tc.tile_pool(name="xlsb", bufs=3) as xlsbp:
                CHUNK = 8
                for jc in range(math.ceil(NXC / CHUNK)):
                    ncols = min(CHUNK * 128, Npad - jc * CHUNK * 128)
                    xtc = xtcp.tile([C, CHUNK * 128], dt.bfloat16)
                    nc.sync.dma_start(
                        xtc[:, :ncols],
                        t_xT[:, jc * CHUNK * 128: jc * CHUNK * 128 + ncols],
                    )
                    xl_sb = xlsbp.tile([128, CHUNK, HD], dt.bfloat16)
                    for j in range(ncols // 128):
                        xl_ps = xlpsp.tile([128, HD], dt.float32, space="PSUM")
                        nc.tensor.matmul(
                            xl_ps[:],
                            xtc[:, j * 128:(j + 1) * 128],
                            wl_sb[:],
                            start=True, stop=True,
                        )
                        if j % 2 == 0:
                            nc.scalar.activation(xl_sb[:, j, :], xl_ps[:],
                                                 AF.Copy)
                        else:
                            nc.vector.tensor_copy(xl_sb[:, j, :], xl_ps[:])
                    row0 = jc * CHUNK * 128
                    nrows = ncols
                    # one batched store per chunk: [128, CHUNK*HD] SBUF ->
                    # row-major [CHUNK*128, HD] DRAM (partition-major blocks)
                    nc.sync.dma_start(
                        t_xl[row0:row0 + nrows, :].rearrange(
                            "(c p) d -> p c d", p=128),
                        xl_sb[:, :nrows // 128, :],
                    )
                xloc = xtcp.tile([C, NLpad], dt.bfloat16, tag="xloc")
                nc.sync.dma_start(xloc[:], t_xT_loc[:, :])
                for b in range(NB):
                    xr_ps = xlpsp.tile([128, HD], dt.float32, space="PSUM")
                    nc.tensor.matmul(
                        xr_ps[:],
                        xloc[:, b * 128:(b + 1) * 128],
                        wr_sb[:],
                        start=True, stop=True,
                    )
                    if has_b:
                        nc.vector.tensor_tensor(
                            out=xr_all[:, b, :], in0=xr_ps[:], in1=bsum_sb[:],
                            op=ALU.add,
                        )
                    else:
                        nc.scalar.activation(xr_all[:, b, :], xr_ps[:], AF.Copy)

            ones_sb = consts.tile([128, NB], dt.float32)
            nc.sync.dma_start(ones_sb[:], t_ones[:, :])
            gamma_sb = consts.tile([D, 1], dt.float32)
            nc.sync.dma_start(gamma_sb[:], t_gamma[:, :])
            beta_sb = consts.tile([D, 1], dt.float32)
            nc.sync.dma_start(beta_sb[:], t_beta[:, :])
            epsp_sb = consts.tile([D, 1], dt.float32)
            nc.sync.dma_start(epsp_sb[:], t_epsp[:, :])

            # ---- main edge loop ----
            from contextlib import ExitStack
            with ExitStack() as es:
                pools = {}
                for nm, bufs, space in [
                    ("gix", 2, None), ("xlg", 2, None), ("xlgT", 2, None),
                    ("st", 2, None), ("sbt", 2, None), ("zps", 3, "PSUM"),
                    ("m", 8, None), ("spack", 1, "PSUM"), ("ee8", 3, None),
                    ("eeps", 1, "PSUM"), ("eesb", 2, None),
                    ("den", 1, "PSUM"), ("rec", 2, None), ("xlw", 8, None),
                    ("ops", 1, "PSUM"), ("post", 2, None),
                ]:
                    kw = {"space": space} if space else {}
                    pools[nm] = es.enter_context(
                        tc.tile_pool(name=nm, bufs=bufs, **kw))
                gixp, gp, gtp, stp, sbtp, zp, mp, spackp, ee8p, eepsp, \
                    eesbp, denp, recp, xlwp, op_, postp = (
                        pools[n] for n in [
                            "gix", "xlg", "xlgT", "st", "sbt", "zps", "m",
                            "spack", "ee8", "eeps", "eesb", "den", "rec",
                            "xlw", "ops", "post"])

                GCH = 4  # tiles per gather (512 idxs)
                # chunk-major packing of the transposed gather: chunk c of
                # gn_c tiles occupies H*gn_c*128 contiguous elems at off_c
                chunk_of = {}
                off = 0
                for g0 in range(0, T, GCH):
                    gn = min(GCH, T - g0)
                    for t in range(g0, g0 + gn):
                        chunk_of[t] = (off, gn, t - g0)
                    off += H * gn * 128

                def emit_heads(b, sbt_sb, xlgT, sp, tg0, tg1, drip=None,
                               per=0):
                    for t in range(tg0, tg1):
                        tl = t - tg0
                        z_ps = zp.tile([128, H, 128], dt.float32,
                                       space="PSUM")
                        off_c, gn_c, tt = chunk_of[t]
                        xlgT_t = bass.AP(
                            tensor=xlgT[:].tensor,
                            offset=xlgT[:].offset + off_c + tt * 128,
                            ap=[xlgT[:].ap[0], [gn_c * 128, H], [1, 128]],
                        )
                        # tiny start=True mm resets the whole bank (PSUM
                        # start semantics are bank-wide); the xr gathers (no
                        # DMA dependency) run next, and the gather-dependent
                        # xlgT add goes last so PE stalls least
                        nc.tensor.matmul(
                            z_ps[:, 0, 0:2], ident_bf[:], zeros2b[:],
                            start=True, stop=False,
                            skip_group_check=True,
                        )
                        for h in range(H):
                            nc.tensor.matmul(
                                z_ps[:, h, :],
                                xr_all[:, b, h * D:(h + 1) * D],
                                sbt_sb[:, t * 128:(t + 1) * 128],
                                start=False, stop=False,
                                skip_group_check=True,
                            )
                        nc.tensor.matmul(
                            z_ps[:], ident_bf[:], xlgT_t,
                            start=False, stop=True,
                            skip_group_check=True,
                        )
                        m_sb = mp.tile([128, H, 128], dt.bfloat16)
                        nc.scalar.activation(
                            m_sb[:], z_ps[:], AF.Prelu, alpha=NEG_SLOPE,
                        )
                        if tl == 0:
                            # att in cols 0-3, zeros after: start=True resets
                            # the whole 64-row stripe pack
                            lhs_att = attP0_sb[:, 0: 4 * GSZ]
                            out_sp = sp[0: 4 * GSZ, :]
                        else:
                            lhs_att = attPZ_sb[:, 4 * GSZ - 4 * tl: 4 * GSZ + 4]
                            out_sp = sp[0: 4 * tl + 4, :]
                        nc.tensor.matmul(
                            out_sp,
                            lhs_att,
                            m_sb[:],
                            start=(tl == 0), stop=(tl == tg1 - tg0 - 1),
                            skip_group_check=True,
                        )
                        for _ in range(per):
                            if drip:
                                drip.pop(0)()

                def make_tails(b, st_sb, xlg, den_ps, out_ps, pee8, p0, p1,
                               epi):
                    """Return a list of closures: transpose+copies, then per
                    tile den-mm + ts/out-mm; finally the block epilogue."""
                    ops = []
                    stride = 4 * GSZ + 1
                    state = {}

                    def t_xpose():
                        # per head-block transpose: ee8[:, h*128:(h+1)*128]
                        # -> eeT_all[:, h, :]; diag (tile tl, head h) then at
                        # flat col h*(4*GSZ+1) + 4*tl
                        eeT_ps = eepsp.tile([128, H, 4 * GSZ], dt.bfloat16,
                                            space="PSUM")
                        for h in range(H):
                            nc.tensor.transpose(
                                eeT_ps[:, h, :],
                                pee8[0: 4 * GSZ, h * 128:(h + 1) * 128],
                                ident_bf[: 4 * GSZ, : 4 * GSZ],
                            )
                        ee_sb = eesbp.tile([128, H * 4 * GSZ], dt.bfloat16)
                        nc.vector.tensor_copy(ee_sb[:], eeT_ps[:])
                        eef = eesbp.tile([128, H * 4 * GSZ], dt.float32,
                                         tag="eef")
                        nc.vector.tensor_copy(eef[:], eeT_ps[:])
                        state["ee_sb"] = ee_sb
                        state["eef"] = eef
                    ops.append(t_xpose)

                    def t_den(t, tl):
                        ee_ap = state["ee_sb"][:]
                        ee_diag = bass.AP(
                            tensor=ee_ap.tensor,
                            offset=ee_ap.offset + 4 * tl,
                            ap=[ee_ap.ap[0], [stride, H]],
                        )
                        nc.tensor.matmul(
                            den_ps[:], st_sb[:, t * 128:(t + 1) * 128],
                            ee_diag,
                            start=(t == 0), stop=(t == T - 1),
                        )

                    def t_out(t, tl):
                        eef = state["eef"]
                        xlw = xlwp.tile([128, HD], dt.bfloat16)
                        for h in range(H):
                            nc.vector.tensor_scalar(
                                out=xlw[:, h * D:(h + 1) * D],
                                in0=xlg[:, t, h * D:(h + 1) * D],
                                scalar1=eef[:, h * stride + 4 * tl:
                                            h * stride + 4 * tl + 1],
                                scalar2=None,
                                op0=ALU.mult,
                            )
                        nc.tensor.matmul(
                            out_ps[:], st_sb[:, t * 128:(t + 1) * 128],
                            xlw[:], start=(t == 0), stop=(t == T - 1),
                        )

                    for t in range(p0, p1):
                        ops.append(lambda t=t, tl=t - p0: t_den(t, tl))
                    for t in range(p0, p1):
                        ops.append(lambda t=t, tl=t - p0: t_out(t, tl))
                    if epi is not None:
                        ops.append(epi)
                    return ops

                def emit_block_epi(b, den_ps, out_ps):
                    rec = recp.tile([128, H], dt.float32)
                    nc.vector.reciprocal(rec[:], den_ps[:])
                    out_sb = postp.tile([128, H, D], dt.float32)
                    rec_ap = rec[:]
                    rec_b = bass.AP(
                        tensor=rec_ap.tensor, offset=rec_ap.offset,
                        ap=[rec_ap.ap[0], rec_ap.ap[1], [0, D]],
                    )
                    nc.vector.tensor_tensor(
                        out=out_sb[:], in0=out_ps[:], in1=rec_b, op=ALU.mult,
                    )
                    if has_b:
                        nc.vector.tensor_tensor(
                            out=out_sb[:], in0=out_sb[:], in1=bl_sb[:],
                            op=ALU.add,
                        )
                    o_ap = out_sb[:]
                    o_swap = bass.AP(   # [128, D, H] view -> reduce heads
                        tensor=o_ap.tensor, offset=o_ap.offset,
                        ap=[o_ap.ap[0], o_ap.ap[2], o_ap.ap[1]],
                    )
                    nc.vector.tensor_reduce(
                        out=om_all[:, b, :], in_=o_swap,
                        axis=mybir.AxisListType.X, op=ALU.add,
                    )
                    sq = postp.tile([128, D], dt.float32)
                    nc.scalar.activation(sq[:], om_all[:, b, :], AF.Square)
                    if b == 0:
                        # full-tile zero reset: a region start=True would
                        # wipe the whole bank (incl. the other column)
                        nc.tensor.matmul(
                            stat_ps[:, 0:2], om_all[:, 0, :], zeros2[:],
                            start=True, stop=False, skip_group_check=True,
                        )
                    nc.tensor.matmul(
                        stat_ps[:, 0:1], om_all[:, b, :],
                        ones_sb[:, b:b + 1],
                        start=False, stop=False,
                        skip_group_check=True,
                    )
                    nc.tensor.matmul(
                        stat_ps[:, 1:2], sq[:],
                        ones_sb[:, b:b + 1],
                        start=False, stop=(b == NB - 1),
                        skip_group_check=True,
                    )

                tailq = []
                for b in range(NB):
                    gix = gixp.tile([128, ET // 16], dt.int16)
                    nc.sync.dma_start(gix[:], t_gidx[b, :, :])
                    sbt_sb = sbtp.tile([128, ET], dt.float8e4)
                    nc.sync.dma_start(sbt_sb[:], t_SbT[b, :, :])
                    xlg = gp.tile([128, T, HD], dt.bfloat16)
                    xlgT = gtp.tile([128, H * ET], dt.bfloat16)
                    # chunk gathers: a single huge dma_gather overflows the
                    # SWDGE descriptor carveout and wedges the device
                    for g0 in range(0, T, GCH):
                        gn = min(GCH, T - g0)
                        off_c = chunk_of[g0][0]
                        xlgT_c = bass.AP(
                            tensor=xlgT[:].tensor,
                            offset=xlgT[:].offset + off_c,
                            ap=[xlgT[:].ap[0], [gn * 128, H], [1, gn * 128]],
                        )
                        nc.gpsimd.dma_gather(
                            xlgT_c, t_xl[:, :],
                            gix[:, g0 * 8:(g0 + gn) * 8],
                            gn * 128, gn * 128, HD,
                            transpose=True,
                        )
                    st_sb = stp.tile([128, ET], dt.float8e4)
                    nc.sync.dma_start(st_sb[:], t_St[b, :, :])
                    for g0 in range(0, T, GCH):
                        gn = min(GCH, T - g0)
                        nc.gpsimd.dma_gather(
                            xlg[:, g0:g0 + gn, :], t_xl[:, :],
                            gix[:, g0 * 8:(g0 + gn) * 8],
                            gn * 128, gn * 128, HD,
                        )

                    den_ps = denp.tile([128, H], dt.float32, space="PSUM")
                    out_ps = op_.tile([128, HD], dt.float32, space="PSUM")

                    ngr = len(groups)
                    for gi, (tg0, tg1) in enumerate(groups):
                        gl = tg1 - tg0
                        sp = spackp.tile([4 * GSZ, 512], dt.float32,
                                         space="PSUM")
                        per = 0
                        emit_heads(b, sbt_sb, xlgT, sp, tg0, tg1,
                                   drip=tailq, per=per)
                        while tailq:
                            tailq.pop(0)()
                        ee8 = ee8p.tile([4 * GSZ, 512], dt.bfloat16)
                        nc.scalar.activation(
                            ee8[: 4 * gl, :], sp[: 4 * gl, :], AF.Exp)
                        epi = None
                        if gi == ngr - 1:
                            epi = (lambda b=b, d=den_ps, o=out_ps:
                                   emit_block_epi(b, d, o))
                        tailq = make_tails(b, st_sb, xlg, den_ps, out_ps,
                                           ee8, tg0, tg1, epi)
                for f in tailq:
                    f()

            # ---- epilogue: BN stats AllReduce, affine, relu, store ----
            with tc.tile_pool(name="epi", bufs=1) as epi, \
                 tc.tile_pool(name="epips", bufs=2, space="PSUM") as epips:
                stat_sb = epi.tile([D, 2], dt.float32)
                nc.scalar.activation(stat_sb[:], stat_ps[:], AF.Copy)
                nc.sync.dma_start(t_ccin[:, :], stat_sb[:])
                if no_cc:
                    nc.sync.dma_start(t_ccout[:, :], t_ccin[:, :])
                else:
                    nc.gpsimd.collective_compute(
                        "AllReduce", ALU.add,
                        replica_groups=[list(range(NCORES))],
                        ins=[t_ccin[:, :].opt()],
                        outs=[t_ccout[:, :].opt()],
                    )
                gst = epi.tile([D, 2], dt.float32)
                nc.sync.dma_start(gst[:], t_ccout[:, :])

                mu = epi.tile([D, 1], dt.float32)
                nc.vector.tensor_scalar(mu[:], gst[:, 0:1], 1.0 / N, None, ALU.mult)
                msq = epi.tile([D, 1], dt.float32)
                nc.vector.tensor_scalar(msq[:], gst[:, 1:2], 1.0 / N, None, ALU.mult)
                var = epi.tile([D, 1], dt.float32)
                nc.vector.tensor_tensor(out=var[:], in0=mu[:], in1=mu[:], op=ALU.mult)
                nc.vector.tensor_tensor(out=var[:], in0=msq[:], in1=var[:],
                                        op=ALU.subtract)
                # rsqrt(var+eps'): ACT Sqrt -> exact reciprocal -> one Newton
                # step (cleans up the sqrt table's loose ULP budget)
                sd = epi.tile([D, 1], dt.float32)
                nc.scalar.activation(sd[:], var[:], AF.Sqrt, bias=epsp_sb[:])
                rs = epi.tile([D, 1], dt.float32)
                nc.vector.reciprocal(rs[:], sd[:])
                vpe = epi.tile([D, 1], dt.float32)
                nc.vector.tensor_tensor(out=vpe[:], in0=var[:], in1=epsp_sb[:],
                                        op=ALU.add)
                r2 = epi.tile([D, 1], dt.float32)
                nc.vector.tensor_tensor(out=r2[:], in0=rs[:], in1=rs[:], op=ALU.mult)
                nc.vector.tensor_tensor(out=r2[:], in0=vpe[:], in1=r2[:], op=ALU.mult)
                nc.vector.tensor_scalar(r2[:], r2[:], -0.5, 1.5, ALU.mult, ALU.add)
                nc.vector.tensor_tensor(out=rs[:], in0=rs[:], in1=r2[:], op=ALU.mult)

                A_col = epi.tile([D, 1], dt.float32)
                nc.vector.tensor_tensor(out=A_col[:], in0=rs[:], in1=gamma_sb[:],
                                        op=ALU.mult)
                B_col = epi.tile([D, 1], dt.float32)
                nc.vector.tensor_tensor(out=B_col[:], in0=mu[:], in1=A_col[:],
                                        op=ALU.mult)
                nc.vector.tensor_tensor(out=B_col[:], in0=beta_sb[:], in1=B_col[:],
                                        op=ALU.subtract)

                a_ps = epips.tile([1, 128], dt.float32, space="PSUM")
                nc.tensor.matmul(a_ps[:], A_col[:],
                                 ident_f32[:], start=True, stop=True)
                b_ps = epips.tile([1, 128], dt.float32, space="PSUM")
                nc.tensor.matmul(b_ps[:], B_col[:],
                                 ident_f32[:], start=True, stop=True)
                a_row = epi.tile([1, 128], dt.float32)
                nc.scalar.activation(a_row[:], a_ps[:], AF.Copy)
                b_row = epi.tile([1, 128], dt.float32)
                nc.scalar.activation(b_row[:], b_ps[:], AF.Copy)
                A_rep = epi.tile([128, 128], dt.float32)
                nc.gpsimd.partition_broadcast(A_rep[:], a_row[:])
                B_rep = epi.tile([128, 128], dt.float32)
                nc.gpsimd.partition_broadcast(B_rep[:], b_row[:])

                with tc.tile_pool(name="yp", bufs=1) as yp:
                    # batched: y = relu(om*A + B) over all NB blocks at once,
                    # A/B broadcast along the block axis via stride-0 views
                    y_all = yp.tile([128, NB, D], dt.float32)
                    A_ap = A_rep[:]
                    A_b = bass.AP(tensor=A_ap.tensor, offset=A_ap.offset,
                                  ap=[A_ap.ap[0], [0, NB], [1, D]])
                    B_ap = B_rep[:]
                    B_b = bass.AP(tensor=B_ap.tensor, offset=B_ap.offset,
                                  ap=[B_ap.ap[0], [0, NB], [1, D]])
                    nc.vector.tensor_tensor(
                        out=y_all[:], in0=om_all[:], in1=A_b, op=ALU.mult,
                    )
                    nc.vector.tensor_tensor(
                        out=y_all[:], in0=y_all[:], in1=B_b, op=ALU.add,
                    )
                    nc.vector.tensor_scalar(
                        y_all[:], y_all[:], 0.0, None, ALU.max,
                    )
                    nc.sync.dma_start(
                        t_y[:, :].rearrange("(c p) d -> p c d", p=128),
                        y_all[:],
                    )

    nc.compile()
    return nc


# --------------------------------------------------------------------------
# Entry point
# --------------------------------------------------------------------------

def kernel(x, edge_index, W_l, b_l, W_r, b_r, att, bias, gamma, beta):
    from concourse.bass_utils import run_bass_kernel_spmd

    hp = _prep_host(x, edge_index, W_l, b_l, W_r, b_r, att, bias, gamma, beta)
    NL = hp["NL"]

    key = (hp["N"], hp["C"], hp["H"], hp["T"], hp["has_b"])
    if key not in _cache:
        _cache[key] = _build_nc(hp)
    nc = _cache[key]

    in_maps = []
    for k in range(NCORES):
        m = dict(
            xT=hp["xT"],
            xT_loc=np.ascontiguousarray(hp["xT_loc"][k]),
            W_l=hp["W_l"], W_r=hp["W_r"],
            attT_col=hp["attT_col"],
            attPad0=hp["attPad0"], attPadZ=hp["attPadZ"],
            gamma_col=hp["gamma_col"], beta_col=hp["beta_col"],
            epsp_col=hp["epsp_col"], ones_m=hp["ones_m"],
            gidx=np.ascontiguousarray(hp["gidx"][k]),
            S_t=np.ascontiguousarray(hp["S_t"][k]),
            S_bT=np.ascontiguousarray(hp["S_bT"][k]),
        )
        if hp["has_b"]:
            m["bsum_rep"] = hp["bsum_rep"]
            m["bl_rep"] = hp["bl_rep"]
        in_maps.append(m)

    res = run_bass_kernel_spmd(nc, in_maps, core_ids=list(range(NCORES)))
    N = hp["N"]
    D = hp["D"]
    out = np.zeros((N, D), np.float32)
    vs = hp["valid_slot"]
    for k in range(NCORES):
        y = res.results[k]["y"]
        out[k * NL + hp["perm"][k][vs]] = y[vs]
    return out
